# revision 6
# baseline (speedup 1.0000x reference)
"""TRN2 Bass kernel for nn_ChartOperator (sparse_attention).

Math (B=4, N=4096, PD=1024, D=16, S=64, ALL=1024):
  P = x @ W_r + b_r
  L = P[..., :ALL].reshape(n, D, S); R = P[..., ALL:].reshape(n, D, S)
  w = softmax_a(L)
  Q[n, d, s] = sum_{a<d} w[n,a,s] * R[n+a+1, d-1-a, s]
  (last D rows of each batch: Q[t+d>=16] zeroed)
  out = Q.reshape(n, ALL) @ W_w + b_w

Sharding: 8 cores, data-parallel over flattened (B*N) rows, 2048 rows/core
plus a 16-row forward halo (zero-padded at batch boundaries; the affected
outputs are exactly the masked ones).

Per-core pipeline (bf16 matmuls, fp32 PSUM), interleaved for engine overlap:
  reader supertiles jj (512 rows each, 0/2 then 1 then 3 then halo) compute
  P transposed into PSUM; ACT exp/copy writes banded-MAC chains
     e chain  et_all[(g2,s64), a16, n1024]   (g: row-blocks 0-7 / 8-15)
     r chain  rt_all[(g2,s64), c15, n1040]   (+ rt_sh, shifted 1 col, so
     every DVE band product is 4B-aligned -> 2x mode)
  softmax Z via chained bf16 adds + reciprocal_approx_fast -> rzb (bf16)
  per chunk cp (128 rows): ebn = e-slice * rz; band products on DVE/GpSimd;
  PE identity-matmuls accumulate into PSUM Q[(g,s), d, n] per 4-d bank;
  per-bank ACT/DVE copies Q -> qt bf16 as soon as each bank closes;
  writer matmuls out[n128, 1024] = qt.T @ W_w -> bf16 DMA out (host adds b_w)
Chunk work for rows covered by supertiles 0/2 is emitted between reader
supertiles so PE / ACT / DVE / GpSimd queues always hold ready work.
"""
import numpy as np
import ml_dtypes
from contextlib import ExitStack

import concourse.bass as bass
import concourse.tile as tile
from concourse import bacc, mybir
from concourse import bass_utils

BF16 = mybir.dt.bfloat16
F32 = mybir.dt.float32
bfnp = ml_dtypes.bfloat16

B, N, PD = 4, 4096, 1024
D, S = 16, 64
ALL = D * S
ROWS_PER_CORE = 2048
HALO = 16
NCP = 8                         # 128-row chunk pairs per chain window

_cache = {}

AF = mybir.ActivationFunctionType
LAST_A = [min(14, 4 * b + 2) for b in range(4)]   # bank b closes after this a


def _build(debug=False):
    nc = bacc.Bacc("TRN2", target_bir_lowering=False, debug=False, num_devices=8)

    # x transposed, per 512-row window: xw[w, ks, k, n]
    xw_d = nc.dram_tensor("xw", [4, 8, 128, 512], BF16, kind="ExternalInput").ap()
    xh_d = nc.dram_tensor("xh", [8, 128, HALO], BF16, kind="ExternalInput").ap()
    wr_d = nc.dram_tensor("wr", [8, 128, 2048], BF16, kind="ExternalInput").ap()
    ww_d = nc.dram_tensor("ww", [8, 128, 1024], BF16, kind="ExternalInput").ap()
    br_d = nc.dram_tensor("br", [128, 16], F32, kind="ExternalInput").ap()
    ident_d = nc.dram_tensor("ident", [128, 128], BF16, kind="ExternalInput").ap()
    qmask_d = nc.dram_tensor("qmask", [128, 8, 128], BF16, kind="ExternalInput").ap()
    out_d = nc.dram_tensor("out", [16, 128, 1024], BF16, kind="ExternalOutput").ap()

    with tile.TileContext(nc) as tc, ExitStack() as ctx:
        cpool = ctx.enter_context(tc.tile_pool(name="cpool", bufs=1))
        xkp = ctx.enter_context(tc.tile_pool(name="xkp", bufs=2))
        ps512 = ctx.enter_context(tc.tile_pool(name="ps512", bufs=2, space="PSUM"))
        wps = ctx.enter_context(tc.tile_pool(name="wps", bufs=2, space="PSUM"))
        macp = ctx.enter_context(tc.tile_pool(name="macp", bufs=1, space="PSUM"))
        zpool = ctx.enter_context(tc.tile_pool(name="zpool", bufs=1))
        prodp = ctx.enter_context(tc.tile_pool(name="prodp", bufs=4))
        prodg = ctx.enter_context(tc.tile_pool(name="prodg", bufs=6))
        qtp = ctx.enter_context(tc.tile_pool(name="qtp", bufs=16))
        osbp = ctx.enter_context(tc.tile_pool(name="osbp", bufs=2))

        # ---- constants / chains (persistent)
        wr_sb = cpool.tile([128, 8, 2048], BF16)
        br_sb = cpool.tile([128, 16], F32)
        ww_sb = cpool.tile([128, 8, 1024], BF16)
        ident = cpool.tile([128, 128], BF16)
        qmask = cpool.tile([128, 8, 128], BF16)
        xh = cpool.tile([128, 8, HALO], BF16)
        et_all = cpool.tile([128, 16, 1024], BF16)   # [(g2,s64), a, n-chain]
        rt_all = cpool.tile([128, 15, 1040], BF16)   # [(g2,s64), c, n-chain]
        rt_sh = cpool.tile([128, 15, 1040], BF16)    # rt_sh[n] = rt_all[n+1]
        zb = cpool.tile([128, 4, 512], BF16)
        zt = cpool.tile([128, 512], F32)
        rz = cpool.tile([128, 512], F32)
        rzb = cpool.tile([128, 512], BF16)           # 1/Z bf16, one window

        # ---- DMA: fine-grained + ordered so the first reader matmul
        # unblocks after ~2 small transfers instead of the whole window.
        nc.sync.dma_start(br_sb[:], br_d[:])
        xk = {}
        xk[0] = xkp.tile([128, 8, 512], BF16, tag="xk", name="xk0")
        for ks in range(8):
            nc.sync.dma_start(wr_sb[:, ks, 0:128], wr_d[ks, :, 0:128])
            nc.gpsimd.dma_start(xk[0][:, ks, :], xw_d[0, ks])
        for ks in range(8):
            nc.sync.dma_start(wr_sb[:, ks, 128:512], wr_d[ks, :, 128:512])
        nc.sync.dma_start(wr_sb[:, :, 512:1024],
                          wr_d[:, :, 512:1024].rearrange("k p c -> p k c"))
        xk[2] = xkp.tile([128, 8, 512], BF16, tag="xk", name="xk2")
        for ks in range(8):
            nc.gpsimd.dma_start(xk[2][:, ks, :], xw_d[2, ks])
        nc.sync.dma_start(wr_sb[:, :, 1024:2048],
                          wr_d[:, :, 1024:2048].rearrange("k p c -> p k c"))
        nc.gpsimd.dma_start(xh[:], xh_d[:].rearrange("k p c -> p k c"))
        nc.sync.dma_start(ww_sb[:], ww_d[:].rearrange("k p c -> p k c"))
        nc.sync.dma_start(ident[:], ident_d[:])
        nc.sync.dma_start(qmask[:], qmask_d[:])

        # ---------------- reader supertile ----------------
        def reader(jj):
            """supertile jj: rows [512*jj, 512*jj+512). g/chain-col mapping:
            jj 0,1 -> g0 cols 512*jj ; jj 2,3 -> g1 cols 512*(jj-2).
            Extra tails: jj2 R rows 1024:1040 -> g0 cols 1024:1040;
            jj4 = halo (16 rows) -> g1 cols 1024:1040, R only."""
            if jj == 4:
                for u in range(8, 16):
                    ps = ps512.tile([128, 512], F32, tag="ps512", name="ps")
                    for ks in range(8):
                        nc.tensor.matmul(ps[:, :HALO], wr_sb[:, ks, 128 * u:128 * (u + 1)],
                                         xh[:, ks, :], start=(ks == 0), stop=(ks == 7))
                    for dsub in range(2):
                        c = 2 * (u - 8) + dsub
                        if c == 15:
                            continue
                        nc.scalar.activation(rt_all[64:128, c, 1024:1024 + HALO],
                                             ps[64 * dsub:64 * dsub + 64, :HALO],
                                             AF.Identity,
                                             bias=br_sb[64 * dsub:64 * dsub + 64, u:u + 1])
                return
            n0 = 512 * jj
            g = 0 if jj < 2 else 1
            lo = 512 * (jj % 2)             # chain col base
            p0 = 64 * g
            for u in range(16):
                is_l = u < 8
                ps = ps512.tile([128, 512], F32, tag="ps512", name="ps")
                for ks in range(8):
                    nc.tensor.matmul(ps[:], wr_sb[:, ks, 128 * u:128 * (u + 1)],
                                     xk[jj][:, ks, :], start=(ks == 0), stop=(ks == 7))
                for dsub in range(2):
                    src = ps[64 * dsub:64 * dsub + 64, :]
                    bias = br_sb[64 * dsub:64 * dsub + 64, u:u + 1]
                    if is_l:
                        a = 2 * u + dsub
                        nc.scalar.activation(et_all[p0:p0 + 64, a, lo:lo + 512],
                                             src, AF.Exp, bias=bias)
                    else:
                        c = 2 * (u - 8) + dsub
                        if c == 15:
                            continue
                        nc.scalar.activation(rt_all[p0:p0 + 64, c, lo:lo + 512],
                                             src, AF.Identity, bias=bias)
                        if jj == 2:   # g0 chain tail rows 1024:1040
                            nc.scalar.activation(rt_all[0:64, c, 1024:1024 + HALO],
                                                 ps[64 * dsub:64 * dsub + 64, :HALO],
                                                 AF.Identity, bias=bias)

        # ---------------- rt_sh shifted-copy regions ----------------
        def rtsh(g, c0, c1):
            """rt_sh[g, :, c0:c1] = rt_all[g, :, c0+1:c1+1] (DVE copy)."""
            p0 = 64 * g
            nc.vector.tensor_copy(rt_sh[p0:p0 + 64, :, c0:c1],
                                  rt_all[p0:p0 + 64, :, c0 + 1:c1 + 1])

        # ---------------- softmax 1/Z for window w (chain cols lo:lo+512) ----
        # split by partition half (g) so the g0 part runs as soon as the g0
        # supertile's L lands, overlapping the other supertile's matmuls
        def softz_g(w, g):
            lo = 512 * w
            p0, p1 = 64 * g, 64 * g + 64
            e = et_all[p0:p1, :, lo:lo + 512]
            nc.vector.tensor_add(zb[p0:p1], e[:, 0:4, :], e[:, 4:8, :])
            nc.vector.tensor_add(zb[p0:p1], zb[p0:p1], e[:, 8:12, :])
            nc.vector.tensor_add(zb[p0:p1], zb[p0:p1], e[:, 12:16, :])
            nc.vector.tensor_add(zb[p0:p1, 0:2, :], zb[p0:p1, 0:2, :],
                                 zb[p0:p1, 2:4, :])
            nc.vector.tensor_add(zt[p0:p1], zb[p0:p1, 0, :], zb[p0:p1, 1, :])

        def softz_fin(w):
            nc.vector.reciprocal_approx_fast(rz[:], zt[:])
            nc.vector.tensor_copy(rzb[:], rz[:])

        def etmul(j):
            """normalize et chain cols [128j, 128j+128) in place (a rows 0:15)"""
            rzs = rzb[:, (128 * j) % 512:(128 * j) % 512 + 128] \
                .rearrange("p (o n) -> p o n", o=1).to_broadcast((128, 15, 128))
            ecol = et_all[:, 0:15, 128 * j:128 * j + 128]
            nc.vector.tensor_mul(ecol, ecol, rzs)

        # ---------------- one 128-row chunk pair ----------------
        GP_A = (10, 11, 12, 13, 14)        # band products routed to GpSimd
        def chunk(cp):
            n0 = 128 * cp
            mp = macp.tile([128, 16, 128], F32, tag="macp", name="mp")
            nc.vector.memset(mp[:, 0, :], 0.0)
            # one qt tile per (g, psum-bank) so writer LDWEIGHTS only depends
            # on the copies of its own bank
            qt = {(g, b): qtp.tile([128, 2, 128], BF16, tag="qtp",
                                   name=f"qt{g}{b}")
                  for g in range(2) for b in range(4)}

            def qt_copy(b):
                """evacuate bank b (qt slabs 2b, 2b+1) once the bank closes"""
                for g in range(2):
                    masked = (cp == 7 and g == 1)
                    for dsub in range(2):
                        csrc = mp[64 * g:64 * g + 64,
                                  4 * b + dsub:4 * b + dsub + 3:2, :]
                        cdst = qt[g, b][64 * dsub:64 * dsub + 64, :, :]
                        if masked:
                            qm = qmask[64 * dsub:64 * dsub + 64, 2 * b:2 * b + 2, :]
                            nc.vector.tensor_mul(cdst, csrc, qm)
                        else:
                            nc.scalar.copy(cdst, csrc)

            for a in range(15):
                cnt = 15 - a
                if a in GP_A:
                    p = prodg.tile([128, 5, 128], BF16, tag="prodg", name="pg")
                    eng = nc.gpsimd
                else:
                    p = prodp.tile([128, 15, 128], BF16, tag="prodp", name="p")
                    eng = nc.vector
                eb = et_all[:, a:a + 1, n0:n0 + 128].to_broadcast((128, cnt, 128))
                if a % 2 == 0:
                    rsrc = rt_sh[:, 0:cnt, n0 + a:n0 + a + 128]
                else:
                    rsrc = rt_all[:, 0:cnt, n0 + a + 1:n0 + a + 129]
                eng.tensor_mul(p[:, 0:cnt, :], eb, rsrc)
                for b in range(4):
                    d_lo = max(a + 1, 4 * b)
                    d_hi = 4 * b + 4
                    if d_lo >= d_hi:
                        continue
                    nc.tensor.matmul(mp[:, d_lo:d_hi, :], ident[:],
                                     p[:, d_lo - a - 1:d_hi - a - 1, :],
                                     start=(a == 0), stop=(a == LAST_A[b]))
                # per-bank evacuation right after the closing matmul
                if a == 2:
                    qt_copy(0)
                elif a == 6:
                    qt_copy(1)
                elif a == 10:
                    qt_copy(2)
                elif a == 14:
                    qt_copy(3)

            for g in range(2):
                cb = 8 * g + cp
                osb = osbp.tile([128, 1024], BF16, tag="osbp", name="osb")
                for h in range(2):
                    wp = wps.tile([128, 512], F32, tag="wps", name="wp")
                    for k in range(8):
                        nc.tensor.matmul(wp[:], qt[g, k // 2][:, k % 2, :],
                                         ww_sb[:, k, h * 512:(h + 1) * 512],
                                         start=(k == 0), stop=(k == 7))
                    nc.scalar.copy(osb[:, h * 512:(h + 1) * 512], wp[:])
                nc.sync.dma_start(out_d[cb], osb[:])

        # ---------------- interleaved program ----------------
        # PE stream: r0, r2, r4, c0, r1, c1, r3, c2, c3..c7 — each chunk's
        # DVE products are emitted one PE-block ahead of their identity MMs.
        reader(0)
        softz_g(0, 0)                        # needs jj0 L only
        rtsh(0, 0, 511)                      # needs jj0 R
        reader(2)
        # prefetch remaining x windows (WAR on jj0/jj2 matmuls via Tile)
        xk[1] = xkp.tile([128, 8, 512], BF16, tag="xk", name="xk1")
        for ks in range(8):
            nc.gpsimd.dma_start(xk[1][:, ks, :], xw_d[1, ks])
        xk[3] = xkp.tile([128, 8, 512], BF16, tag="xk", name="xk3")
        for ks in range(8):
            nc.gpsimd.dma_start(xk[3][:, ks, :], xw_d[3, ks])
        reader(4)                            # halo: covers PE while softz runs
        softz_g(0, 1)                        # needs jj2 L
        softz_fin(0)
        etmul(0)
        rtsh(1, 0, 160)                      # needs jj2 R -> unblocks cp0
        chunk(0)
        etmul(1)
        etmul(2)
        etmul(3)
        rtsh(1, 160, 511)
        rtsh(0, 1023, 1039)                  # jj2 g0 tail
        reader(1)
        chunk(1)
        rtsh(0, 511, 1023)                   # needs jj1 R
        reader(3)
        chunk(2)
        rtsh(1, 511, 1023)                   # needs jj3 R
        softz_g(1, 0)                        # needs jj1 L
        softz_g(1, 1)                        # needs jj3 L
        softz_fin(1)
        etmul(4)
        etmul(5)
        etmul(6)
        etmul(7)
        rtsh(1, 1023, 1039)                  # needs jj4 halo R
        chunk(3)
        chunk(4)
        chunk(5)
        chunk(6)
        chunk(7)

    nc.compile()
    return nc


def _host_prep(x, W_r, b_r, W_w, b_w):
    """Build the 8 per-core input maps."""
    xf = np.asarray(x, np.float32).reshape(B * N, PD)
    wr = np.asarray(W_r, np.float32).astype(bfnp)
    ww = np.asarray(W_w, np.float32).astype(bfnp)
    br = np.ascontiguousarray(
        np.asarray(b_r, np.float32).reshape(16, 128).T)
    wr_t = np.ascontiguousarray(wr.reshape(8, 128, 2048))
    ww_t = np.ascontiguousarray(ww.reshape(8, 128, 1024))
    ident = np.eye(128, dtype=np.float32).astype(bfnp)

    in_maps = []
    for c in range(8):
        lo = c * ROWS_PER_CORE
        chunk = np.zeros((ROWS_PER_CORE + HALO, PD), np.float32)
        chunk[:ROWS_PER_CORE] = xf[lo:lo + ROWS_PER_CORE]
        if c % 2 == 0:
            chunk[ROWS_PER_CORE:] = xf[lo + ROWS_PER_CORE: lo + ROWS_PER_CORE + HALO]
        cb = chunk.astype(bfnp)
        # xw[w, ks, k, n] = chunk[512*w + n, 128*ks + k]
        xw = np.ascontiguousarray(
            cb[:ROWS_PER_CORE].reshape(4, 512, 8, 128).transpose(0, 2, 3, 1))
        xh = np.ascontiguousarray(
            cb[ROWS_PER_CORE:].reshape(HALO, 8, 128).transpose(1, 2, 0))
        qmask = np.ones((128, 8, 128), np.float32)
        if c % 2 == 1:
            dsub = (np.arange(128)[:, None, None] // 64)
            k = np.arange(8)[None, :, None]
            n = np.arange(128)[None, None, :]
            bad = (n >= 112) & ((n - 112 + 2 * k + dsub) >= 16)
            qmask[np.broadcast_to(bad, (128, 8, 128))] = 0.0
        in_maps.append({
            "xw": xw, "xh": xh,
            "wr": wr_t, "ww": ww_t, "br": br,
            "ident": ident, "qmask": qmask.astype(bfnp),
        })
    return in_maps


def kernel(x, W_r, b_r, W_w, b_w):
    if "nc" not in _cache:
        _cache["nc"] = _build()
    nc = _cache["nc"]
    in_maps = _host_prep(x, W_r, b_r, W_w, b_w)
    res = bass_utils.run_bass_kernel_spmd(nc, in_maps, core_ids=list(range(8)))
    out = np.concatenate([np.asarray(r["out"], np.float32)
                          .reshape(ROWS_PER_CORE, ALL)
                          for r in res.results], axis=0)
    out = out.reshape(B, N, ALL)
    out += np.asarray(b_w, np.float32).reshape(1, 1, ALL)
    return np.ascontiguousarray(out)



# revision 9
# speedup vs baseline: 1.0178x; 1.0178x over previous
"""TRN2 Bass kernel for nn_ChartOperator (sparse_attention).

Math (B=4, N=4096, PD=1024, D=16, S=64, ALL=1024):
  P = x @ W_r + b_r
  L = P[..., :ALL].reshape(n, D, S); R = P[..., ALL:].reshape(n, D, S)
  w = softmax_a(L)
  Q[n, d, s] = sum_{a<d} w[n,a,s] * R[n+a+1, d-1-a, s]
  (last D rows of each batch: Q[t+d>=16] zeroed)
  out = Q.reshape(n, ALL) @ W_w + b_w

Sharding: 8 cores, data-parallel over flattened (B*N) rows, 2048 rows/core
plus a 16-row forward halo (zero-padded at batch boundaries; the affected
outputs are exactly the masked ones).

Per-core pipeline (bf16 matmuls, fp32 PSUM), interleaved for engine overlap:
  reader supertiles jj (512 rows each, 0/2 then 1 then 3 then halo) compute
  P transposed into PSUM; ACT exp/copy writes banded-MAC chains
     e chain  et_all[(g2,s64), a16, n1024]   (g: row-blocks 0-7 / 8-15)
     r chain  rt_all[(g2,s64), c15, n1040]   (+ rt_sh, shifted 1 col, so
     every DVE band product is 4B-aligned -> 2x mode)
  softmax Z via chained bf16 adds + reciprocal_approx_fast -> rzb (bf16)
  per chunk cp (128 rows): ebn = e-slice * rz; band products on DVE/GpSimd;
  PE identity-matmuls accumulate into PSUM Q[(g,s), d, n] per 4-d bank;
  per-bank ACT/DVE copies Q -> qt bf16 as soon as each bank closes;
  writer matmuls out[n128, 1024] = qt.T @ W_w -> bf16 DMA out (host adds b_w)
Chunk work for rows covered by supertiles 0/2 is emitted between reader
supertiles so PE / ACT / DVE / GpSimd queues always hold ready work.
"""
import numpy as np
import ml_dtypes
from contextlib import ExitStack

import concourse.bass as bass
import concourse.tile as tile
from concourse import bacc, mybir
from concourse import bass_utils

BF16 = mybir.dt.bfloat16
F32 = mybir.dt.float32
bfnp = ml_dtypes.bfloat16

B, N, PD = 4, 4096, 1024
D, S = 16, 64
ALL = D * S
ROWS_PER_CORE = 2048
HALO = 16
NCP = 8                         # 128-row chunk pairs per chain window

_cache = {}

AF = mybir.ActivationFunctionType
LAST_A = [min(14, 4 * b + 2) for b in range(4)]   # bank b closes after this a


def _build(debug=False):
    nc = bacc.Bacc("TRN2", target_bir_lowering=False, debug=False, num_devices=8)

    # x transposed, per 512-row window: xw[w, ks, k, n]
    xw_d = nc.dram_tensor("xw", [4, 8, 128, 512], BF16, kind="ExternalInput").ap()
    xh_d = nc.dram_tensor("xh", [8, 128, HALO], BF16, kind="ExternalInput").ap()
    wr_d = nc.dram_tensor("wr", [8, 128, 2048], BF16, kind="ExternalInput").ap()
    ww_d = nc.dram_tensor("ww", [8, 128, 1024], BF16, kind="ExternalInput").ap()
    br_d = nc.dram_tensor("br", [128, 16], F32, kind="ExternalInput").ap()
    ident_d = nc.dram_tensor("ident", [128, 128], BF16, kind="ExternalInput").ap()
    qmask_d = nc.dram_tensor("qmask", [128, 8, 128], BF16, kind="ExternalInput").ap()
    out_d = nc.dram_tensor("out", [16, 128, 1024], BF16, kind="ExternalOutput").ap()

    with tile.TileContext(nc) as tc, ExitStack() as ctx:
        cpool = ctx.enter_context(tc.tile_pool(name="cpool", bufs=1))
        xkp = ctx.enter_context(tc.tile_pool(name="xkp", bufs=2))
        ps512 = ctx.enter_context(tc.tile_pool(name="ps512", bufs=2, space="PSUM"))
        wps = ctx.enter_context(tc.tile_pool(name="wps", bufs=2, space="PSUM"))
        macp = ctx.enter_context(tc.tile_pool(name="macp", bufs=1, space="PSUM"))
        zpool = ctx.enter_context(tc.tile_pool(name="zpool", bufs=1))
        prodp = ctx.enter_context(tc.tile_pool(name="prodp", bufs=4))
        prodg = ctx.enter_context(tc.tile_pool(name="prodg", bufs=6))
        qtp = ctx.enter_context(tc.tile_pool(name="qtp", bufs=16))
        osbp = ctx.enter_context(tc.tile_pool(name="osbp", bufs=2))

        # ---- constants / chains (persistent)
        wr_sb = cpool.tile([128, 8, 2048], BF16)
        br_sb = cpool.tile([128, 16], F32)
        ww_sb = cpool.tile([128, 8, 1024], BF16)
        ident = cpool.tile([128, 128], BF16)
        qmask = cpool.tile([128, 8, 128], BF16)
        xh = cpool.tile([128, 8, HALO], BF16)
        et_all = cpool.tile([128, 16, 1024], BF16)   # [(g2,s64), a, n-chain]
        rt_all = cpool.tile([128, 15, 1040], BF16)   # [(g2,s64), c, n-chain]
        rt_sh = cpool.tile([128, 15, 1040], BF16)    # rt_sh[n] = rt_all[n+1]
        zb = cpool.tile([128, 4, 512], BF16)
        zt = cpool.tile([128, 512], F32)
        rz = cpool.tile([128, 512], F32)
        rzb = cpool.tile([128, 512], BF16)           # 1/Z bf16, one window

        # ---- DMA: fine-grained + ordered so the first reader matmul
        # unblocks after ~2 small transfers instead of the whole window.
        nc.sync.dma_start(br_sb[:], br_d[:])
        xk = {}
        xk[0] = xkp.tile([128, 8, 512], BF16, tag="xk", name="xk0")
        for ks in range(8):
            nc.sync.dma_start(wr_sb[:, ks, 0:128], wr_d[ks, :, 0:128])
            nc.gpsimd.dma_start(xk[0][:, ks, :], xw_d[0, ks])
        for ks in range(8):
            nc.sync.dma_start(wr_sb[:, ks, 128:512], wr_d[ks, :, 128:512])
        nc.sync.dma_start(wr_sb[:, :, 512:1024],
                          wr_d[:, :, 512:1024].rearrange("k p c -> p k c"))
        xk[2] = xkp.tile([128, 8, 512], BF16, tag="xk", name="xk2")
        for ks in range(8):
            nc.gpsimd.dma_start(xk[2][:, ks, :], xw_d[2, ks])
        nc.sync.dma_start(wr_sb[:, :, 1024:2048],
                          wr_d[:, :, 1024:2048].rearrange("k p c -> p k c"))
        nc.gpsimd.dma_start(xh[:], xh_d[:].rearrange("k p c -> p k c"))
        nc.sync.dma_start(ww_sb[:], ww_d[:].rearrange("k p c -> p k c"))
        nc.sync.dma_start(ident[:], ident_d[:])
        nc.sync.dma_start(qmask[:], qmask_d[:])

        # ---------------- reader supertile ----------------
        def reader(jj):
            """supertile jj: rows [512*jj, 512*jj+512). g/chain-col mapping:
            jj 0,1 -> g0 cols 512*jj ; jj 2,3 -> g1 cols 512*(jj-2).
            Extra tails: jj2 R rows 1024:1040 -> g0 cols 1024:1040;
            jj4 = halo (16 rows) -> g1 cols 1024:1040, R only."""
            if jj == 4:
                for u in range(8, 16):
                    ps = ps512.tile([128, 512], F32, tag="ps512", name="ps")
                    for ks in range(8):
                        nc.tensor.matmul(ps[:, :HALO], wr_sb[:, ks, 128 * u:128 * (u + 1)],
                                         xh[:, ks, :], start=(ks == 0), stop=(ks == 7))
                    for dsub in range(2):
                        c = 2 * (u - 8) + dsub
                        if c == 15:
                            continue
                        nc.scalar.activation(rt_all[64:128, c, 1024:1024 + HALO],
                                             ps[64 * dsub:64 * dsub + 64, :HALO],
                                             AF.Identity,
                                             bias=br_sb[64 * dsub:64 * dsub + 64, u:u + 1])
                return
            n0 = 512 * jj
            g = 0 if jj < 2 else 1
            lo = 512 * (jj % 2)             # chain col base
            p0 = 64 * g
            for u in range(16):
                is_l = u < 8
                ps = ps512.tile([128, 512], F32, tag="ps512", name="ps")
                for ks in range(8):
                    nc.tensor.matmul(ps[:], wr_sb[:, ks, 128 * u:128 * (u + 1)],
                                     xk[jj][:, ks, :], start=(ks == 0), stop=(ks == 7))
                for dsub in range(2):
                    src = ps[64 * dsub:64 * dsub + 64, :]
                    bias = br_sb[64 * dsub:64 * dsub + 64, u:u + 1]
                    if is_l:
                        a = 2 * u + dsub
                        nc.scalar.activation(et_all[p0:p0 + 64, a, lo:lo + 512],
                                             src, AF.Exp, bias=bias)
                    else:
                        c = 2 * (u - 8) + dsub
                        if c == 15:
                            continue
                        nc.scalar.activation(rt_all[p0:p0 + 64, c, lo:lo + 512],
                                             src, AF.Identity, bias=bias)
                        if jj == 2:   # g0 chain tail rows 1024:1040
                            nc.scalar.activation(rt_all[0:64, c, 1024:1024 + HALO],
                                                 ps[64 * dsub:64 * dsub + 64, :HALO],
                                                 AF.Identity, bias=bias)

        # ---------------- rt_sh shifted-copy regions ----------------
        def rtsh(g, c0, c1):
            """rt_sh[g, :, c0:c1] = rt_all[g, :, c0+1:c1+1] (DVE copy)."""
            p0 = 64 * g
            nc.vector.tensor_copy(rt_sh[p0:p0 + 64, :, c0:c1],
                                  rt_all[p0:p0 + 64, :, c0 + 1:c1 + 1])

        # ---------------- softmax 1/Z for window w (chain cols lo:lo+512) ----
        # split by partition half (g) so the g0 part runs as soon as the g0
        # supertile's L lands, overlapping the other supertile's matmuls
        def softz_g(w, g):
            lo = 512 * w
            p0, p1 = 64 * g, 64 * g + 64
            e = et_all[p0:p1, :, lo:lo + 512]
            nc.vector.tensor_add(zb[p0:p1], e[:, 0:4, :], e[:, 4:8, :])
            nc.vector.tensor_add(zb[p0:p1], zb[p0:p1], e[:, 8:12, :])
            nc.vector.tensor_add(zb[p0:p1], zb[p0:p1], e[:, 12:16, :])
            nc.vector.tensor_add(zb[p0:p1, 0:2, :], zb[p0:p1, 0:2, :],
                                 zb[p0:p1, 2:4, :])
            nc.vector.tensor_add(zt[p0:p1], zb[p0:p1, 0, :], zb[p0:p1, 1, :])

        def softz_fin(w):
            nc.vector.reciprocal_approx_fast(rz[:], zt[:])
            nc.vector.tensor_copy(rzb[:], rz[:])

        def etmul(j):
            """normalize et chain cols [128j, 128j+128) in place (a rows 0:15)"""
            rzs = rzb[:, (128 * j) % 512:(128 * j) % 512 + 128] \
                .rearrange("p (o n) -> p o n", o=1).to_broadcast((128, 15, 128))
            ecol = et_all[:, 0:15, 128 * j:128 * j + 128]
            nc.vector.tensor_mul(ecol, ecol, rzs)

        # ---------------- one 128-row chunk pair ----------------
        GP_A = (11, 12, 13, 14)            # band products routed to GpSimd
        def chunk(cp):
            n0 = 128 * cp
            mp = macp.tile([128, 16, 128], F32, tag="macp", name="mp")
            nc.vector.memset(mp[:, 0, :], 0.0)
            # one qt tile per (g, psum-bank) so writer LDWEIGHTS only depends
            # on the copies of its own bank
            qt = {(g, b): qtp.tile([128, 2, 128], BF16, tag="qtp",
                                   name=f"qt{g}{b}")
                  for g in range(2) for b in range(4)}

            def qt_copy(b):
                """evacuate bank b (qt slabs 2b, 2b+1) once the bank closes.
                bank 3 goes on DVE: it closes last (a=14) and gates the
                writer's final LDWEIGHTS, so the faster engine trims the
                PE bubble between identity MMs and writer MMs."""
                for g in range(2):
                    masked = (cp == 7 and g == 1)
                    for dsub in range(2):
                        csrc = mp[64 * g:64 * g + 64,
                                  4 * b + dsub:4 * b + dsub + 3:2, :]
                        cdst = qt[g, b][64 * dsub:64 * dsub + 64, :, :]
                        if masked:
                            qm = qmask[64 * dsub:64 * dsub + 64, 2 * b:2 * b + 2, :]
                            nc.vector.tensor_mul(cdst, csrc, qm)
                        elif b == 3:
                            nc.vector.tensor_copy(cdst, csrc)
                        else:
                            nc.scalar.copy(cdst, csrc)

            for a in range(15):
                cnt = 15 - a
                if a in GP_A:
                    p = prodg.tile([128, 4, 128], BF16, tag="prodg", name="pg")
                    eng = nc.gpsimd
                else:
                    p = prodp.tile([128, 15, 128], BF16, tag="prodp", name="p")
                    eng = nc.vector
                eb = et_all[:, a:a + 1, n0:n0 + 128].to_broadcast((128, cnt, 128))
                if a % 2 == 0:
                    rsrc = rt_sh[:, 0:cnt, n0 + a:n0 + a + 128]
                else:
                    rsrc = rt_all[:, 0:cnt, n0 + a + 1:n0 + a + 129]
                eng.tensor_mul(p[:, 0:cnt, :], eb, rsrc)
                for b in range(4):
                    d_lo = max(a + 1, 4 * b)
                    d_hi = 4 * b + 4
                    if d_lo >= d_hi:
                        continue
                    nc.tensor.matmul(mp[:, d_lo:d_hi, :], ident[:],
                                     p[:, d_lo - a - 1:d_hi - a - 1, :],
                                     start=(a == 0), stop=(a == LAST_A[b]))
                # per-bank evacuation right after the closing matmul
                if a == 2:
                    qt_copy(0)
                elif a == 6:
                    qt_copy(1)
                elif a == 10:
                    qt_copy(2)
                elif a == 14:
                    qt_copy(3)

            for g in range(2):
                cb = 8 * g + cp
                osb = osbp.tile([128, 1024], BF16, tag="osbp", name="osb")
                for h in range(2):
                    wp = wps.tile([128, 512], F32, tag="wps", name="wp")
                    for k in range(8):
                        nc.tensor.matmul(wp[:], qt[g, k // 2][:, k % 2, :],
                                         ww_sb[:, k, h * 512:(h + 1) * 512],
                                         start=(k == 0), stop=(k == 7))
                    nc.scalar.copy(osb[:, h * 512:(h + 1) * 512], wp[:])
                nc.sync.dma_start(out_d[cb], osb[:])

        # ---------------- interleaved program ----------------
        # PE stream: r0, r2, r4, c0, r1, c1, r3, c2, c3..c7 — each chunk's
        # DVE products are emitted one PE-block ahead of their identity MMs.
        reader(0)
        softz_g(0, 0)                        # needs jj0 L only
        rtsh(0, 0, 511)                      # needs jj0 R
        reader(2)
        # prefetch remaining x windows (WAR on jj0/jj2 matmuls via Tile)
        xk[1] = xkp.tile([128, 8, 512], BF16, tag="xk", name="xk1")
        for ks in range(8):
            nc.gpsimd.dma_start(xk[1][:, ks, :], xw_d[1, ks])
        xk[3] = xkp.tile([128, 8, 512], BF16, tag="xk", name="xk3")
        for ks in range(8):
            nc.gpsimd.dma_start(xk[3][:, ks, :], xw_d[3, ks])
        reader(4)                            # halo: covers PE while softz runs
        softz_g(0, 1)                        # needs jj2 L
        softz_fin(0)
        etmul(0)
        rtsh(1, 0, 160)                      # needs jj2 R -> unblocks cp0
        chunk(0)
        etmul(1)
        etmul(2)
        etmul(3)
        rtsh(1, 160, 511)
        rtsh(0, 1023, 1039)                  # jj2 g0 tail
        reader(1)
        chunk(1)
        rtsh(0, 511, 1023)                   # needs jj1 R
        reader(3)
        chunk(2)
        rtsh(1, 511, 1023)                   # needs jj3 R
        softz_g(1, 0)                        # needs jj1 L
        softz_g(1, 1)                        # needs jj3 L
        softz_fin(1)
        etmul(4)
        etmul(5)
        etmul(6)
        etmul(7)
        rtsh(1, 1023, 1039)                  # needs jj4 halo R
        chunk(3)
        chunk(4)
        chunk(5)
        chunk(6)
        chunk(7)

    nc.compile()
    return nc


def _host_prep(x, W_r, b_r, W_w, b_w):
    """Build the 8 per-core input maps."""
    xf = np.asarray(x, np.float32).reshape(B * N, PD)
    wr = np.asarray(W_r, np.float32).astype(bfnp)
    ww = np.asarray(W_w, np.float32).astype(bfnp)
    br = np.ascontiguousarray(
        np.asarray(b_r, np.float32).reshape(16, 128).T)
    wr_t = np.ascontiguousarray(wr.reshape(8, 128, 2048))
    ww_t = np.ascontiguousarray(ww.reshape(8, 128, 1024))
    ident = np.eye(128, dtype=np.float32).astype(bfnp)

    in_maps = []
    for c in range(8):
        lo = c * ROWS_PER_CORE
        chunk = np.zeros((ROWS_PER_CORE + HALO, PD), np.float32)
        chunk[:ROWS_PER_CORE] = xf[lo:lo + ROWS_PER_CORE]
        if c % 2 == 0:
            chunk[ROWS_PER_CORE:] = xf[lo + ROWS_PER_CORE: lo + ROWS_PER_CORE + HALO]
        cb = chunk.astype(bfnp)
        # xw[w, ks, k, n] = chunk[512*w + n, 128*ks + k]
        xw = np.ascontiguousarray(
            cb[:ROWS_PER_CORE].reshape(4, 512, 8, 128).transpose(0, 2, 3, 1))
        xh = np.ascontiguousarray(
            cb[ROWS_PER_CORE:].reshape(HALO, 8, 128).transpose(1, 2, 0))
        qmask = np.ones((128, 8, 128), np.float32)
        if c % 2 == 1:
            dsub = (np.arange(128)[:, None, None] // 64)
            k = np.arange(8)[None, :, None]
            n = np.arange(128)[None, None, :]
            bad = (n >= 112) & ((n - 112 + 2 * k + dsub) >= 16)
            qmask[np.broadcast_to(bad, (128, 8, 128))] = 0.0
        in_maps.append({
            "xw": xw, "xh": xh,
            "wr": wr_t, "ww": ww_t, "br": br,
            "ident": ident, "qmask": qmask.astype(bfnp),
        })
    return in_maps


def kernel(x, W_r, b_r, W_w, b_w):
    if "nc" not in _cache:
        _cache["nc"] = _build()
    nc = _cache["nc"]
    in_maps = _host_prep(x, W_r, b_r, W_w, b_w)
    res = bass_utils.run_bass_kernel_spmd(nc, in_maps, core_ids=list(range(8)))
    out = np.concatenate([np.asarray(r["out"], np.float32)
                          .reshape(ROWS_PER_CORE, ALL)
                          for r in res.results], axis=0)
    out = out.reshape(B, N, ALL)
    out += np.asarray(b_w, np.float32).reshape(1, 1, ALL)
    return np.ascontiguousarray(out)

